# revision 1
# baseline (speedup 1.0000x reference)
"""GNN message-passing (SAGE-gcn + GraphConv stack) Trainium2 Bass kernel.

Strategy (8 NeuronCores, dst-node sharded):
- Each core owns a contiguous block of N/8 dst nodes and all edges into them.
- Per layer, the small weight matmul is applied BEFORE aggregation
  (A @ (h W^T) == (A @ h) W^T), shrinking the gathered message dim.
- Each core computes z for its own nodes; an in-kernel AllGather replicates
  the full [N, Dz] z table (bf16) into every core's DRAM.
- Aggregation: edges sorted by dst; per 128-dst tile, dma_gather pulls
  z[src] rows (edges on partitions), a one-hot S matrix (built on-device by
  iota==dstloc compare) turns segment-sum into PE matmuls accumulating in
  PSUM (exact fp32 accumulation of bf16 messages).
- Host does index-only preprocessing: edge bucketing/sorting, int16 gather
  indices (split in two 25k src halves to fit int16), degree scalars.
"""
import os
import sys
import time
import hashlib

import numpy as np
import ml_dtypes

for _p in ("/opt/trn_rl_repo", "/root/.axon_site/_ro/trn_rl_repo"):
    if os.path.isdir(_p) and _p not in sys.path:
        sys.path.append(_p)

import concourse.bass as bass  # noqa: E402
import concourse.bacc as bacc  # noqa: E402
import concourse.mybir as mybir  # noqa: E402
from concourse import tile  # noqa: E402
from concourse import bass_utils  # noqa: E402

BF16 = mybir.dt.bfloat16
F32 = mybir.dt.float32
I16 = mybir.dt.int16
I32 = mybir.dt.int32

NCORES = 8
PT = 128  # partition/tile size


# ----------------------------------------------------------------------------
# host-side graph preprocessing (index-only)
# ----------------------------------------------------------------------------

def _wrap_idxs(idx_flat: np.ndarray) -> np.ndarray:
    """[n] int16 -> [128, n/16]: idx i at [i%16, i//16], replicated x8."""
    n = idx_flat.shape[0]
    assert n % 16 == 0
    w = idx_flat.reshape(n // 16, 16).T.astype(np.int16)
    return np.ascontiguousarray(np.tile(w, (8, 1)))


def _prep_graph(src: np.ndarray, dst: np.ndarray, n_nodes: int):
    """Bucket edges by (core, dst-tile, src-half); build uniform chunk table.

    Returns (meta, per_core) where meta has the core-uniform structure and
    per_core has the int16 gather indices + dstloc arrays for each core.
    """
    nshard = n_nodes // NCORES
    nt = (nshard + PT - 1) // PT
    half_n = ((n_nodes + 1) // 2)
    assert half_n - 1 <= np.iinfo(np.int16).max

    src = np.asarray(src, np.int64)
    dst = np.asarray(dst, np.int64)

    counts = np.zeros((NCORES, nt, 2), np.int64)
    per_core_sorted = []
    for c in range(NCORES):
        mask = (dst >= c * nshard) & (dst < (c + 1) * nshard)
        es = src[mask]
        ed = dst[mask] - c * nshard
        t = ed >> 7
        m = ed & 127
        h = (es >= half_n).astype(np.int64)
        order = np.lexsort((es, h, t))
        es, m, t, h = es[order], m[order], t[order], h[order]
        key = t * 2 + h
        cnt = np.bincount(key, minlength=nt * 2).reshape(nt, 2)
        counts[c] = cnt
        per_core_sorted.append((es, m))

    cchunks = (counts + PT - 1) // PT  # per-core chunks per (t, h)
    c_th = cchunks.max(axis=0)  # [nt, 2] uniform chunk counts
    offs = np.zeros((nt, 2), np.int64)
    acc = 0
    for t in range(nt):
        for h in range(2):
            offs[t, h] = acc
            acc += c_th[t, h]
    tc = int(acc)  # total chunks

    per_core = []
    for c in range(NCORES):
        es, m = per_core_sorted[c]
        idx_flat = np.zeros(tc * PT, np.int16)
        dstloc = np.full((PT, tc), -1.0, np.float32)
        pos = 0
        for t in range(nt):
            for h in range(2):
                n = int(counts[c, t, h])
                o = int(offs[t, h]) * PT
                ii = es[pos:pos + n] - h * half_n
                idx_flat[o:o + n] = ii.astype(np.int16)
                mm = m[pos:pos + n]
                col = (np.arange(n) // PT) + int(offs[t, h])
                row = np.arange(n) % PT
                dstloc[row, col] = mm.astype(np.float32)
                pos += n
        per_core.append(dict(
            idx=_wrap_idxs(idx_flat),
            dstloc=np.ascontiguousarray(dstloc.astype(ml_dtypes.bfloat16)),
        ))

    meta = dict(nshard=nshard, nt=nt, half_n=half_n, tc=tc,
                c_th=c_th, offs=offs)
    return meta, per_core


# ----------------------------------------------------------------------------
# device program builder
# ----------------------------------------------------------------------------

def _build_program(meta, dims, fc_out):
    """dims: list of (Din, Dout) for the 3 encoder stages.
    Aggregation layers: sage(D0out), gc(D0out), sage(D1out), gc(D1out),
    sage(D2out), gc(D2out); then fc.
    """
    nshard, nt, tc = meta["nshard"], meta["nt"], meta["tc"]
    c_th, offs = meta["c_th"], meta["offs"]
    half_n = meta["half_n"]
    din0 = dims[0][0]
    # timing-bisection variants (numerically invalid when nonzero)
    v_gather = os.environ.get("GNN_V_GATHER", "gather")  # gather|contig|skip
    v_aggmm = int(os.environ.get("GNN_V_AGGMM", "1"))    # 0: one mm per tile
    v_s = int(os.environ.get("GNN_V_S", "1"))            # 0: skip S gen
    maxc = int(os.environ.get("GNN_MAXC", "8"))
    nqueues = int(os.environ.get("GNN_QUEUES", "1"))
    # weight list applied at stage B k: k=0 -> ws0, 1 -> wg0, ... 6 -> fc
    wdims = []
    for (di, do) in dims:
        wdims.append((di, do))   # sage
        wdims.append((do, do))   # gc
    wdims.append((dims[-1][1], fc_out))  # fc
    agg_dz = [wdims[k][1] for k in range(6)]  # dim aggregated at agg layer k

    nc = bacc.Bacc("TRN2", target_bir_lowering=False, debug=False,
                   num_devices=NCORES, num_swdge_queues=nqueues)

    feat = nc.dram_tensor("feat", [nt * PT, din0], BF16, kind="ExternalInput")
    idx_in = nc.dram_tensor("idx", [PT, tc * 8], I16, kind="ExternalInput")
    dstloc_in = nc.dram_tensor("dstloc", [PT, tc], BF16, kind="ExternalInput")
    iota_in = nc.dram_tensor("iota", [PT, PT], BF16, kind="ExternalInput")
    ident_in = nc.dram_tensor("ident", [PT, PT], BF16, kind="ExternalInput")
    invden_in = nc.dram_tensor("invden", [PT, nt], F32, kind="ExternalInput")
    invdst_in = nc.dram_tensor("invdst", [PT, nt], F32, kind="ExternalInput")
    invsrc_in = nc.dram_tensor("invsrc", [PT, nt], F32, kind="ExternalInput")
    w_in = []
    for k, (di, do) in enumerate(wdims):
        w_in.append(nc.dram_tensor(f"w{k}", [PT, di // PT, do], BF16,
                                   kind="ExternalInput"))
    out = nc.dram_tensor("out", [nshard, fc_out], F32, kind="ExternalOutput")

    rows = [PT] * nt
    rows[nt - 1] = nshard - (nt - 1) * PT

    with tile.TileContext(nc) as tc_ctx:
        tcx = tc_ctx
        with (
            tcx.tile_pool(name="const", bufs=1) as constp,
            tcx.tile_pool(name="g0", bufs=2) as g0p,
            tcx.tile_pool(name="g1", bufs=2) as g1p,
            tcx.tile_pool(name="s", bufs=2) as sp,
            tcx.tile_pool(name="h", bufs=3) as hp,
            tcx.tile_pool(name="ht", bufs=2) as htp,
            tcx.tile_pool(name="zo", bufs=3) as zop,
            tcx.tile_pool(name="u", bufs=2) as up,
            tcx.tile_pool(name="zown", bufs=2) as zownp,
            tcx.tile_pool(name="aggps", bufs=2, space="PSUM") as aggpsp,
            tcx.tile_pool(name="trps", bufs=2, space="PSUM") as trpsp,
            tcx.tile_pool(name="mmps", bufs=2, space="PSUM") as mmpsp,
            tcx.tile_pool(name="dram", bufs=1, space="DRAM") as dramp,
        ):
            # persistent constants
            idx_sb = constp.tile([PT, tc * 8], I16, name="idx_sb")
            nc.sync.dma_start(idx_sb[:], idx_in[:])
            dstloc_sb = constp.tile([PT, tc], BF16, name="dstloc_sb")
            nc.sync.dma_start(dstloc_sb[:], dstloc_in[:])
            iota_sb = constp.tile([PT, PT], BF16, name="iota_sb")
            nc.sync.dma_start(iota_sb[:], iota_in[:])
            ident_sb = constp.tile([PT, PT], BF16, name="ident_sb")
            nc.sync.dma_start(ident_sb[:], ident_in[:])
            invden_sb = constp.tile([PT, nt], F32, name="invden_sb")
            nc.sync.dma_start(invden_sb[:], invden_in[:])
            invdst_sb = constp.tile([PT, nt], F32, name="invdst_sb")
            nc.sync.dma_start(invdst_sb[:], invdst_in[:])
            invsrc_sb = constp.tile([PT, nt], F32, name="invsrc_sb")
            nc.sync.dma_start(invsrc_sb[:], invsrc_in[:])
            w_sb = []
            for k, (di, do) in enumerate(wdims):
                w = constp.tile([PT, di // PT, do], BF16, name=f"w{k}_sb")
                nc.sync.dma_start(w[:], w_in[k][:])
                w_sb.append(w)

            # internal DRAM: z shards + gathered full tables
            def alloc_zbufs(rep):
                z_own, z_full = [], []
                for a in range(6):
                    dz = agg_dz[a]
                    z_own.append(dramp.tile([nshard, dz], BF16,
                                            name=f"zown{a}_r{rep}"))
                    z_full.append(dramp.tile([NCORES * nshard, dz], BF16,
                                             name=f"zfull{a}_r{rep}",
                                             addr_space="Shared"))
                return z_own, z_full

            def stage_b(t, h_tile, k, dest, dest_dtype, prescale, final):
                """h_tile [128, Din_k] bf16 -> dest rows (t). k = weight idx."""
                di, do = wdims[k]
                kg = di // PT
                ht = htp.tile([PT, kg, PT], BF16, tag="ht")
                for g in range(kg):
                    trp = trpsp.tile([PT, PT], BF16, tag="tr")
                    nc.tensor.transpose(trp[:], h_tile[:, g * PT:(g + 1) * PT],
                                        ident_sb[:])
                    nc.vector.tensor_copy(ht[:, g, :], trp[:])
                zp = mmpsp.tile([PT, do], F32, tag="mm")
                for g in range(kg):
                    nc.tensor.matmul(zp[:], ht[:, g, :], w_sb[k][:, g, :],
                                     start=(g == 0), stop=(g == kg - 1))
                zo = zop.tile([PT, do], dest_dtype, tag="zo")
                if prescale is not None:
                    nc.scalar.activation(zo[:], zp[:],
                                         mybir.ActivationFunctionType.Copy,
                                         scale=prescale[:, t:t + 1])
                else:
                    nc.vector.tensor_copy(zo[:], zp[:])
                nc.sync.dma_start(dest[t * PT:t * PT + rows[t], :],
                                  zo[:rows[t], :])

            v_ag = int(os.environ.get("GNN_V_AG", "1"))

            def run_ag(a):
                if not v_ag:
                    return
                nc.gpsimd.collective_compute(
                    "AllGather", mybir.AluOpType.bypass,
                    replica_groups=[list(range(NCORES))],
                    ins=[z_own[a][:].opt()],
                    outs=[z_full[a][:].opt()],
                )

            # ---- stage B0: features -> z0 ----
            n_repeat = int(os.environ.get("GNN_REPEAT", "1"))
            for _rep in range(n_repeat):
              z_own, z_full = alloc_zbufs(_rep)
              for t in range(nt):
                h0 = hp.tile([PT, din0], BF16, tag="h")
                nc.sync.dma_start(h0[:], feat[t * PT:(t + 1) * PT, :])
                stage_b(t, h0, 0, z_own[0], BF16, None, False)
              run_ag(0)

              # ---- 6 aggregation layers + following stage B ----
              for a in range(6):
                dz = agg_dz[a]
                kind = "sage" if a % 2 == 0 else "gc"
                k_next = a + 1
                is_fc = (k_next == 6)
                dnext = wdims[k_next][1]
                for t in range(nt):
                    # gathers (per src half)
                    gh = []
                    for hh, gp in ((0, g0p), (1, g1p)):
                        cth = int(c_th[t, hh])
                        if cth == 0:
                            continue
                        off = int(offs[t, hh])
                        g = gp.tile([PT, cth, dz], BF16, tag=f"g{hh}")
                        half = z_full[a][hh * half_n:
                                         hh * half_n + (half_n if hh == 0 else
                                                        NCORES * nshard - half_n), :]
                        for c0 in range(0, cth, maxc):
                            cw = min(maxc, cth - c0)
                            if v_gather == "gather":
                                nc.gpsimd.dma_gather(
                                    g[:, c0:c0 + cw, :], half,
                                    idx_sb[:, (off + c0) * 8:(off + c0 + cw) * 8],
                                    cw * PT, cw * PT, dz,
                                    queue_num=(off + c0) % nqueues)
                            elif v_gather == "contig":
                                nc.sync.dma_start(
                                    g[:, c0:c0 + cw, :],
                                    half[(off + c0) * PT % 8192:
                                         (off + c0) * PT % 8192 + cw * PT, :]
                                    .rearrange("(c p) d -> p c d", p=PT))
                            # skip: emit nothing
                        gh.append((g, cth))
                    ct = sum(c for _, c in gh)
                    off0 = int(offs[t, 0])
                    s = sp.tile([PT, ct, PT], BF16, tag="s")
                    if v_s:
                        iota_b = iota_sb[:].unsqueeze(1) \
                            .broadcast_to([PT, ct, PT])
                        dl_b = dstloc_sb[:, off0:off0 + ct].unsqueeze(2) \
                            .broadcast_to([PT, ct, PT])
                        nc.vector.tensor_tensor(s[:], iota_b, dl_b,
                                                mybir.AluOpType.is_equal)
                    aggp = aggpsp.tile([PT, dz], F32, tag="agg")
                    ci = 0
                    for g, cth in gh:
                        for cc in range(cth):
                            if not v_aggmm and not (ci == 0 or ci == ct - 1):
                                ci += 1
                                continue
                            nc.tensor.matmul(aggp[:], s[:, ci, :], g[:, cc, :],
                                             start=(ci == 0),
                                             stop=(ci == ct - 1))
                            ci += 1
                    # epilogue
                    h = hp.tile([PT, dz], BF16, tag="h")
                    if kind == "sage":
                        zt = zownp.tile([PT, dz], F32, tag="zown")
                        if rows[t] < PT:
                            nc.vector.memset(zt[:], 0.0)
                        # SWDGE dma casts bf16 -> f32
                        nc.gpsimd.dma_start(
                            zt[:rows[t], :],
                            z_own[a][t * PT:t * PT + rows[t], :])
                        u = up.tile([PT, dz], F32, tag="u")
                        nc.vector.tensor_tensor(u[:], aggp[:], zt[:],
                                                mybir.AluOpType.add)
                        nc.scalar.activation(h[:], u[:],
                                             mybir.ActivationFunctionType.Relu,
                                             scale=invden_sb[:, t:t + 1])
                    else:
                        nc.scalar.activation(h[:], aggp[:],
                                             mybir.ActivationFunctionType.Relu,
                                             scale=invdst_sb[:, t:t + 1])
                    # next weight stage
                    if is_fc:
                        stage_b(t, h, 6, out, F32, None, True)
                    else:
                        prescale = invsrc_sb if kind == "sage" else None
                        stage_b(t, h, k_next, z_own[k_next], BF16, prescale,
                                False)
                if not is_fc:
                    run_ag(k_next)

    t0 = time.time()
    nc.compile()
    print(f"[kernel] bacc compile: {time.time() - t0:.1f}s", file=sys.stderr)
    return nc


# ----------------------------------------------------------------------------
# public entry
# ----------------------------------------------------------------------------

_CACHE = {}


def _build_in_maps(features, src, dst, sage_ws, gc_ws, fc_w, meta, per_core):
    n_nodes, din0 = features.shape
    dims = [(w.shape[1], w.shape[0]) for w in sage_ws]
    nshard, nt = meta["nshard"], meta["nt"]

    # degree scalars (host, index-only)
    e_ones = np.ones(len(src), np.float64)
    in_deg = np.bincount(dst, weights=e_ones, minlength=n_nodes)
    out_deg = np.bincount(src, weights=e_ones, minlength=n_nodes)
    inv_den = (1.0 / (in_deg + 1.0)).astype(np.float32)
    inv_dst = (np.where(in_deg > 0, in_deg, 1.0) ** -0.5).astype(np.float32)
    inv_src = (np.where(out_deg > 0, out_deg, 1.0) ** -0.5).astype(np.float32)

    def shard_scal(v, c):
        s = np.ones(nt * PT, np.float32)
        s[:nshard] = v[c * nshard:(c + 1) * nshard]
        return np.ascontiguousarray(s.reshape(nt, PT).T)

    # weights: w [Dout, Din] -> wT tiles [128, Din/128, Dout] bf16
    worder = []
    for s in range(len(dims)):
        worder.append(sage_ws[s])
        worder.append(gc_ws[s])
    worder.append(fc_w)
    w_arrs = []
    for w in worder:
        do, di = w.shape
        wt = np.ascontiguousarray(
            w.T.astype(np.float32).reshape(di // PT, PT, do)
            .transpose(1, 0, 2)).astype(ml_dtypes.bfloat16)
        w_arrs.append(wt)

    iota = np.tile(np.arange(PT, dtype=np.float32), (PT, 1)) \
        .astype(ml_dtypes.bfloat16)
    ident = np.eye(PT, dtype=np.float32).astype(ml_dtypes.bfloat16)

    in_maps = []
    for c in range(NCORES):
        fpad = np.zeros((nt * PT, din0), np.float32)
        fpad[:nshard] = features[c * nshard:(c + 1) * nshard]
        im = dict(
            feat=fpad.astype(ml_dtypes.bfloat16),
            idx=per_core[c]["idx"],
            dstloc=per_core[c]["dstloc"],
            iota=iota,
            ident=ident,
            invden=shard_scal(inv_den, c),
            invdst=shard_scal(inv_dst, c),
            invsrc=shard_scal(inv_src, c),
        )
        for k, w in enumerate(w_arrs):
            im[f"w{k}"] = w
        in_maps.append(im)
    return in_maps


def _run(features, src, dst, sage_ws, sage_bs, gc_ws, gc_bs, fc_w, fc_b):
    n_nodes, din0 = features.shape
    dims = [(w.shape[1], w.shape[0]) for w in sage_ws]
    fc_out = fc_w.shape[0]

    key = hashlib.sha1(
        np.asarray(src).tobytes() + np.asarray(dst).tobytes()
        + str((n_nodes, din0, dims, fc_out)).encode()
    ).hexdigest()
    if key in _CACHE:
        nc, meta, per_core = _CACHE[key]
    else:
        meta, per_core = _prep_graph(src, dst, n_nodes)
        nc = _build_program(meta, dims, fc_out)
        _CACHE[key] = (nc, meta, per_core)

    biases = list(sage_bs) + list(gc_bs) + [fc_b]
    if any(np.any(np.asarray(b) != 0) for b in biases):
        raise NotImplementedError("nonzero biases not supported")

    in_maps = _build_in_maps(features, src, dst, sage_ws, gc_ws, fc_w,
                             meta, per_core)

    trace = bool(int(os.environ.get("GNN_TRACE", "0")))
    res = bass_utils.run_bass_kernel_spmd(
        nc, in_maps, core_ids=list(range(NCORES)), trace=trace)
    out = np.concatenate([res.results[c]["out"] for c in range(NCORES)],
                         axis=0).astype(np.float32)
    if trace:
        print(f"[kernel] exec_time_ns: {res.exec_time_ns}", file=sys.stderr)
        _CACHE["last_exec_time_ns"] = res.exec_time_ns
        _CACHE["last_profile"] = res.profile_json
    return out


def kernel(features, src, dst,
           sage_w0, sage_b0, gc_w0, gc_b0,
           sage_w1, sage_b1, gc_w1, gc_b1,
           sage_w2, sage_b2, gc_w2, gc_b2,
           fc_w, fc_b):
    features = np.asarray(features, np.float32)
    src = np.asarray(src, np.int64)
    dst = np.asarray(dst, np.int64)
    return _run(
        features, src, dst,
        [np.asarray(sage_w0), np.asarray(sage_w1), np.asarray(sage_w2)],
        [np.asarray(sage_b0), np.asarray(sage_b1), np.asarray(sage_b2)],
        [np.asarray(gc_w0), np.asarray(gc_w1), np.asarray(gc_w2)],
        [np.asarray(gc_b0), np.asarray(gc_b1), np.asarray(gc_b2)],
        np.asarray(fc_w), np.asarray(fc_b),
    )



# revision 4
# speedup vs baseline: 1.2956x; 1.2956x over previous
"""GNN message-passing (SAGE-gcn + GraphConv stack) Trainium2 Bass kernel. V2

Strategy (8 NeuronCores, dst-node sharded):
- Each core owns a contiguous block of N/8 dst nodes and all edges into them.
- Per layer, the small weight matmul is applied BEFORE aggregation
  (A @ (h W^T) == (A @ h) W^T), shrinking the gathered message dim.
- Each core computes z for its own nodes; an in-kernel AllGather replicates
  the full [N, Dz] z table (bf16) into every core's DRAM.
- Aggregation: edges sorted by (src-half, dst-tile); gathered in uniform
  1024-row dma_gather calls spread over 4 SWDGE queues (SWDGE descriptor
  generation is the bottleneck resource). Chunks of 128 gathered rows are
  segment-summed into per-dst-tile PSUM via one-hot S matmuls; chunks may
  straddle dst-tile boundaries (one matmul per (chunk, tile) "job").
- Self-loops are removed from the gather stream. The sage "+h" term and the
  self-loop contribution ride a per-tile "local chunk" (contiguous HWDGE load
  of the core's own z rows) multiplied by an identity S column (twice for
  sage: 2*I accounts for self-loop + explicit h).
- Duplicate (src, dst-tile) edges are deduped: one gathered row can serve two
  dst positions via a second one-hot layer (d2) added to S on DVE.
- Host does index-only preprocessing: edge bucketing, dedup, int16 gather
  indices (two 25k src halves for int16 range), degree scalars.
"""
import os
import sys
import time
import hashlib

import numpy as np
import ml_dtypes

for _p in ("/opt/trn_rl_repo", "/root/.axon_site/_ro/trn_rl_repo"):
    if os.path.isdir(_p) and _p not in sys.path:
        sys.path.append(_p)

import concourse.bass as bass  # noqa: E402
import concourse.bacc as bacc  # noqa: E402
import concourse.mybir as mybir  # noqa: E402
from concourse import tile  # noqa: E402
from concourse import bass_utils  # noqa: E402

BF16 = mybir.dt.bfloat16
F32 = mybir.dt.float32
I16 = mybir.dt.int16

NCORES = 8
PT = 128
NQUEUES = 4
MAXC = 8  # chunks (x128 rows) per dma_gather call; 1024-idx HW cap


# ----------------------------------------------------------------------------
# host-side graph preprocessing (index-only)
# ----------------------------------------------------------------------------

def _wrap_idxs(idx_flat: np.ndarray) -> np.ndarray:
    """[n] int16 -> [128, n/16]: idx i at [i%16, i//16], replicated x8."""
    n = idx_flat.shape[0]
    assert n % 16 == 0
    w = idx_flat.reshape(n // 16, 16).T.astype(np.int16)
    return np.ascontiguousarray(np.tile(w, (8, 1)))


def _prep_graph(src: np.ndarray, dst: np.ndarray, n_nodes: int):
    """Bucket/dedup edges; build the uniform chunk/job layout.

    Returns (meta, per_core):
      meta: nshard, nt, half_n, nch=[c0,c1], jobs_by_tile (uniform),
            njobs, rows_ut [2][nt]
      per_core: idx [128, (c0+c1)*8] int16, d1/d2 [128, njobs] bf16
    """
    nshard = n_nodes // NCORES
    nt = (nshard + PT - 1) // PT
    half_n = n_nodes // 2
    assert half_n - 1 <= np.iinfo(np.int16).max

    src = np.asarray(src, np.int64)
    dst = np.asarray(dst, np.int64)
    nonself = src != dst
    src, dst = src[nonself], dst[nonself]

    # per core: rows[(h, t)] -> list of (src_local, m1, m2)
    core_rows = []
    for c in range(NCORES):
        mask = (dst >= c * nshard) & (dst < (c + 1) * nshard)
        es = src[mask]
        ed = dst[mask] - c * nshard
        h = (es >= half_n).astype(np.int64)
        t = ed >> 7
        m = ed & 127
        order = np.lexsort((m, es, t, h))
        es, t, m, h = es[order], t[order], m[order], h[order]
        rows = {}
        i = 0
        E = len(es)
        while i < E:
            # group run of identical (h, t, src)
            j = i + 1
            while j < E and es[j] == es[i] and t[j] == t[i] and h[j] == h[i]:
                j += 1
            key = (int(h[i]), int(t[i]))
            lst = rows.setdefault(key, [])
            sl = int(es[i] - h[i] * half_n)
            k = i
            while k + 1 < j:  # pairs
                lst.append((sl, int(m[k]), int(m[k + 1])))
                k += 2
            if k < j:
                lst.append((sl, int(m[k]), -1))
            i = j
        core_rows.append(rows)

    # uniform per-cell row counts
    rows_ut = np.zeros((2, nt), np.int64)
    for h in range(2):
        for t in range(nt):
            rows_ut[h, t] = max(len(core_rows[c].get((h, t), []))
                                for c in range(NCORES))

    # chunk layout per half; tile 48 region extended to chunk pad
    off = np.zeros((2, nt), np.int64)
    nch = [0, 0]
    rows_ext = rows_ut.copy()
    for h in range(2):
        acc = 0
        for t in range(nt):
            off[h, t] = acc
            acc += rows_ut[h, t]
        nch[h] = (acc + PT - 1) // PT
        rows_ext[h, nt - 1] += nch[h] * PT - acc  # pad into last tile

    # jobs: per tile, ordered (h asc, chunk asc). job -> (h, c, t)
    jobs_by_tile = []
    njobs = 0
    for t in range(nt):
        jt = []
        for h in range(2):
            lo, hi = off[h, t], off[h, t] + rows_ext[h, t]
            for c in range(int(lo) >> 7, int(hi - 1 >> 7) + 1):
                jt.append((h, c, njobs))
                njobs += 1
        jobs_by_tile.append(jt)

    # per-core tables
    per_core = []
    for c in range(NCORES):
        idx_flat = np.zeros((nch[0] + nch[1]) * PT, np.int16)
        d1 = np.full((PT, njobs), -1.0, np.float32)
        d2 = np.full((PT, njobs), -1.0, np.float32)
        rowbuf = {}
        for h in range(2):
            for t in range(nt):
                lst = core_rows[c].get((h, t), [])
                base = (nch[0] * PT if h else 0) + int(off[h, t])
                for r, (sl, m1, m2) in enumerate(lst):
                    idx_flat[base + r] = sl
                    rowbuf[base + r] = (m1, m2)
        for t in range(nt):
            for (h, ch, j) in jobs_by_tile[t]:
                gbase = (nch[0] * PT if h else 0) + ch * PT
                lo = (nch[0] * PT if h else 0) + int(off[h, t])
                hi = lo + int(rows_ut[h, t])
                for p in range(PT):
                    g = gbase + p
                    if lo <= g < hi and g in rowbuf:
                        m1, m2 = rowbuf[g]
                        d1[p, j] = float(m1)
                        if m2 >= 0:
                            d2[p, j] = float(m2)
        per_core.append(dict(
            idx=_wrap_idxs(idx_flat),
            d1=np.ascontiguousarray(d1.astype(ml_dtypes.bfloat16)),
            d2=np.ascontiguousarray(d2.astype(ml_dtypes.bfloat16)),
        ))

    meta = dict(nshard=nshard, nt=nt, half_n=half_n, nch=nch,
                jobs_by_tile=jobs_by_tile, njobs=njobs, rows_ut=rows_ut)
    return meta, per_core


# ----------------------------------------------------------------------------
# device program builder
# ----------------------------------------------------------------------------

def _build_program(meta, dims, fc_out):
    nshard, nt, half_n = meta["nshard"], meta["nt"], meta["half_n"]
    nch = meta["nch"]
    jobs_by_tile, njobs = meta["jobs_by_tile"], meta["njobs"]
    nchunks = nch[0] + nch[1]
    din0 = dims[0][0]

    wdims = []
    for (di, do) in dims:
        wdims.append((di, do))   # sage
        wdims.append((do, do))   # gc
    wdims.append((dims[-1][1], fc_out))  # fc
    agg_dz = [wdims[k][1] for k in range(6)]
    max_dz = max(agg_dz)
    max_jobs_t = max(len(j) for j in jobs_by_tile)

    rows = [PT] * nt
    rows[nt - 1] = nshard - (nt - 1) * PT
    npad = nt * PT  # padded z_own rows

    nc = bacc.Bacc("TRN2", target_bir_lowering=False, debug=False,
                   num_devices=NCORES, num_swdge_queues=NQUEUES)

    feat = nc.dram_tensor("feat", [npad, din0], BF16, kind="ExternalInput")
    idx_in = nc.dram_tensor("idx", [PT, nchunks * 8], I16,
                            kind="ExternalInput")
    d1_in = nc.dram_tensor("d1", [PT, njobs], BF16, kind="ExternalInput")
    d2_in = nc.dram_tensor("d2", [PT, njobs], BF16, kind="ExternalInput")
    iota_in = nc.dram_tensor("iota", [PT, PT], BF16, kind="ExternalInput")
    ident_in = nc.dram_tensor("ident", [PT, PT], BF16, kind="ExternalInput")
    dloc_in = nc.dram_tensor("dloc", [PT, nt], BF16, kind="ExternalInput")
    invden_in = nc.dram_tensor("invden", [PT, nt], F32, kind="ExternalInput")
    invdst_in = nc.dram_tensor("invdst", [PT, nt], F32, kind="ExternalInput")
    invsrc_in = nc.dram_tensor("invsrc", [PT, nt], F32, kind="ExternalInput")
    w_in = []
    for k, (di, do) in enumerate(wdims):
        w_in.append(nc.dram_tensor(f"w{k}", [PT, di // PT, do], BF16,
                                   kind="ExternalInput"))
    out = nc.dram_tensor("out", [nshard, fc_out], F32, kind="ExternalOutput")

    with tile.TileContext(nc) as tcx:
        with (
            tcx.tile_pool(name="const", bufs=1) as constp,
            tcx.tile_pool(name="g0", bufs=3) as g0p,
            tcx.tile_pool(name="g1", bufs=3) as g1p,
            tcx.tile_pool(name="zt", bufs=2) as ztp,
            tcx.tile_pool(name="s", bufs=2) as sp,
            tcx.tile_pool(name="s2", bufs=2) as s2p,
            tcx.tile_pool(name="s3", bufs=2) as s3p,
            tcx.tile_pool(name="h", bufs=3) as hp,
            tcx.tile_pool(name="ht", bufs=2) as htp,
            tcx.tile_pool(name="zo", bufs=3) as zop,
            tcx.tile_pool(name="aggps", bufs=2, space="PSUM") as aggpsp,
            tcx.tile_pool(name="trps", bufs=2, space="PSUM") as trpsp,
            tcx.tile_pool(name="mmps", bufs=2, space="PSUM") as mmpsp,
            tcx.tile_pool(name="dram", bufs=1, space="DRAM") as dramp,
        ):
            # persistent constants
            idx_sb = constp.tile([PT, nchunks * 8], I16, name="idx_sb")
            nc.sync.dma_start(idx_sb[:], idx_in[:])
            d1_sb = constp.tile([PT, njobs], BF16, name="d1_sb")
            nc.sync.dma_start(d1_sb[:], d1_in[:])
            d2_sb = constp.tile([PT, njobs], BF16, name="d2_sb")
            nc.sync.dma_start(d2_sb[:], d2_in[:])
            iota_sb = constp.tile([PT, PT], BF16, name="iota_sb")
            nc.sync.dma_start(iota_sb[:], iota_in[:])
            ident_sb = constp.tile([PT, PT], BF16, name="ident_sb")
            nc.sync.dma_start(ident_sb[:], ident_in[:])
            dloc_sb = constp.tile([PT, nt], BF16, name="dloc_sb")
            nc.sync.dma_start(dloc_sb[:], dloc_in[:])
            invden_sb = constp.tile([PT, nt], F32, name="invden_sb")
            nc.sync.dma_start(invden_sb[:], invden_in[:])
            invdst_sb = constp.tile([PT, nt], F32, name="invdst_sb")
            nc.sync.dma_start(invdst_sb[:], invdst_in[:])
            invsrc_sb = constp.tile([PT, nt], F32, name="invsrc_sb")
            nc.sync.dma_start(invsrc_sb[:], invsrc_in[:])
            w_sb = []
            for k, (di, do) in enumerate(wdims):
                w = constp.tile([PT, di // PT, do], BF16, name=f"w{k}_sb")
                nc.sync.dma_start(w[:], w_in[k][:])
                w_sb.append(w)

            # per-tile local identity columns: ls[:, t, :] = I masked to rows[t]
            ls_sb = constp.tile([PT, nt, PT], BF16, name="ls_sb")
            iota_bnt = iota_sb[:].unsqueeze(1).broadcast_to([PT, nt, PT])
            dloc_bnt = dloc_sb[:].unsqueeze(2).broadcast_to([PT, nt, PT])
            nc.vector.tensor_tensor(ls_sb[:], iota_bnt, dloc_bnt,
                                    mybir.AluOpType.is_equal)

            # internal DRAM: z shards (padded) + gathered full tables
            z_own, z_full = [], []
            for a in range(6):
                dz = agg_dz[a]
                z_own.append(dramp.tile([npad, dz], BF16, name=f"zown{a}"))
                z_full.append(dramp.tile([NCORES * nshard, dz], BF16,
                                         name=f"zfull{a}",
                                         addr_space="Shared"))

            def run_ag(a):
                nc.gpsimd.collective_compute(
                    "AllGather", mybir.AluOpType.bypass,
                    replica_groups=[list(range(NCORES))],
                    ins=[z_own[a][:nshard, :].opt()],
                    outs=[z_full[a][:].opt()],
                )

            def stage_b(t, h_tile, k, dest, dest_dtype, prescale):
                """h_tile [128, Din_k] bf16 -> dest rows of tile t."""
                di, do = wdims[k]
                kg = di // PT
                ht = htp.tile([PT, kg, PT], BF16, tag="ht")
                for g in range(kg):
                    trp = trpsp.tile([PT, PT], BF16, tag="tr")
                    nc.tensor.transpose(trp[:], h_tile[:, g * PT:(g + 1) * PT],
                                        ident_sb[:])
                    nc.vector.tensor_copy(ht[:, g, :], trp[:])
                zp = mmpsp.tile([PT, do], F32, tag="mm")
                for g in range(kg):
                    nc.tensor.matmul(zp[:], ht[:, g, :], w_sb[k][:, g, :],
                                     start=(g == 0), stop=(g == kg - 1))
                zo = zop.tile([PT, do], dest_dtype, tag="zo")
                if prescale is not None:
                    nc.scalar.activation(zo[:], zp[:],
                                         mybir.ActivationFunctionType.Copy,
                                         scale=prescale[:, t:t + 1])
                else:
                    nc.vector.tensor_copy(zo[:], zp[:])
                if dest_dtype == F32:  # fc output: unpadded dest
                    nc.sync.dma_start(dest[t * PT:t * PT + rows[t], :],
                                      zo[:rows[t], :])
                else:
                    nc.sync.dma_start(dest[t * PT:(t + 1) * PT, :], zo[:])

            # ---- stage B0: features -> z0 ----
            for t in range(nt):
                h0 = hp.tile([PT, din0], BF16, tag="h")
                nc.sync.dma_start(h0[:], feat[t * PT:(t + 1) * PT, :])
                stage_b(t, h0, 0, z_own[0], BF16, None)
            run_ag(0)

            # ---- 6 aggregation layers + following stage B ----
            call_id = 0
            for a in range(6):
                dz = agg_dz[a]
                sage = (a % 2 == 0)
                k_next = a + 1
                is_fc = (k_next == 6)

                gbufs = [{}, {}]  # per half: call -> g tile
                cursor = [0, 0]   # next call to emit, per half
                ncalls = [(nch[0] + MAXC - 1) // MAXC,
                          (nch[1] + MAXC - 1) // MAXC]

                def emit_calls(h, upto_chunk):
                    nonlocal call_id
                    gp = g0p if h == 0 else g1p
                    half = z_full[a][h * half_n:(h + 1) * half_n, :]
                    while (cursor[h] * MAXC <= upto_chunk
                           and cursor[h] < ncalls[h]):
                        ci = cursor[h]
                        cw = min(MAXC, nch[h] - ci * MAXC)
                        g = gp.tile([PT, MAXC, dz], BF16, tag=f"g{h}")
                        base = (nch[0] if h else 0) + ci * MAXC
                        nc.gpsimd.dma_gather(
                            g[:, :cw, :], half,
                            idx_sb[:, base * 8:(base + cw) * 8],
                            cw * PT, cw * PT, dz,
                            queue_num=call_id % NQUEUES)
                        gbufs[h][ci] = g
                        cursor[h] += 1
                        call_id += 1

                for t in range(nt):
                    jt = jobs_by_tile[t]
                    nj = len(jt)
                    j0 = jt[0][2]
                    assert [j for (_, _, j) in jt] == list(range(j0, j0 + nj))
                    for h in (0, 1):
                        hc = [c for (hh, c, _) in jt if hh == h]
                        if hc:
                            emit_calls(h, max(hc))
                    # local chunk (self-loop + sage's explicit +h)
                    zt = ztp.tile([PT, dz], BF16, tag="zt")
                    nc.sync.dma_start(zt[:],
                                      z_own[a][t * PT:(t + 1) * PT, :])
                    # S for gather jobs: eq(d1) + eq(d2)
                    s = sp.tile([PT, nj, PT], BF16, tag="s")
                    iota_b = iota_sb[:].unsqueeze(1).broadcast_to([PT, nj, PT])
                    d1_b = d1_sb[:, j0:j0 + nj].unsqueeze(2) \
                        .broadcast_to([PT, nj, PT])
                    nc.vector.tensor_tensor(s[:], iota_b, d1_b,
                                            mybir.AluOpType.is_equal)
                    s2 = s2p.tile([PT, nj, PT], BF16, tag="s2")
                    d2_b = d2_sb[:, j0:j0 + nj].unsqueeze(2) \
                        .broadcast_to([PT, nj, PT])
                    nc.vector.tensor_tensor(s2[:], iota_b, d2_b,
                                            mybir.AluOpType.is_equal)
                    s3 = s3p.tile([PT, nj, PT], BF16, tag="s3")
                    nc.vector.tensor_tensor(s3[:], s[:], s2[:],
                                            mybir.AluOpType.add)
                    # matmuls: local first (x2 for sage), then gather jobs
                    aggp = aggpsp.tile([PT, dz], F32, tag="agg")
                    nc.tensor.matmul(aggp[:], ls_sb[:, t, :], zt[:],
                                     start=True, stop=False)
                    if sage:
                        nc.tensor.matmul(aggp[:], ls_sb[:, t, :], zt[:],
                                         start=False, stop=False)
                    for ji, (h, c, j) in enumerate(jt):
                        g = gbufs[h][c // MAXC]
                        nc.tensor.matmul(aggp[:], s3[:, ji, :],
                                         g[:, c % MAXC, :],
                                         start=False, stop=(ji == nj - 1))
                    # epilogue: relu(scale * agg)
                    hx = hp.tile([PT, dz], BF16, tag="h")
                    scl = invden_sb if sage else invdst_sb
                    nc.scalar.activation(hx[:], aggp[:],
                                         mybir.ActivationFunctionType.Relu,
                                         scale=scl[:, t:t + 1])
                    # next weight stage
                    if is_fc:
                        stage_b(t, hx, 6, out, F32, None)
                    else:
                        prescale = invsrc_sb if sage else None
                        stage_b(t, hx, k_next, z_own[k_next], BF16, prescale)
                if not is_fc:
                    run_ag(k_next)

    t0 = time.time()
    nc.compile()
    print(f"[kernel] bacc compile: {time.time() - t0:.1f}s", file=sys.stderr)
    return nc


# ----------------------------------------------------------------------------
# public entry
# ----------------------------------------------------------------------------

_CACHE = {}


def _build_in_maps(features, src, dst, sage_ws, gc_ws, fc_w, meta, per_core):
    n_nodes, din0 = features.shape
    nshard, nt = meta["nshard"], meta["nt"]

    e_ones = np.ones(len(src), np.float64)
    in_deg = np.bincount(dst, weights=e_ones, minlength=n_nodes)
    out_deg = np.bincount(src, weights=e_ones, minlength=n_nodes)
    inv_den = (1.0 / (in_deg + 1.0)).astype(np.float32)
    inv_dst = (np.where(in_deg > 0, in_deg, 1.0) ** -0.5).astype(np.float32)
    inv_src = (np.where(out_deg > 0, out_deg, 1.0) ** -0.5).astype(np.float32)

    def shard_scal(v, c):
        s = np.ones(nt * PT, np.float32)
        s[:nshard] = v[c * nshard:(c + 1) * nshard]
        return np.ascontiguousarray(s.reshape(nt, PT).T)

    worder = []
    for s in range(len(sage_ws)):
        worder.append(sage_ws[s])
        worder.append(gc_ws[s])
    worder.append(fc_w)
    w_arrs = []
    for w in worder:
        do, di = w.shape
        wt = np.ascontiguousarray(
            w.T.astype(np.float32).reshape(di // PT, PT, do)
            .transpose(1, 0, 2)).astype(ml_dtypes.bfloat16)
        w_arrs.append(wt)

    iota = np.tile(np.arange(PT, dtype=np.float32), (PT, 1)) \
        .astype(ml_dtypes.bfloat16)
    ident = np.eye(PT, dtype=np.float32).astype(ml_dtypes.bfloat16)
    # dloc[:, t] = partition index if < rows[t] else -1
    rows = [PT] * nt
    rows[nt - 1] = nshard - (nt - 1) * PT
    dloc = np.full((PT, nt), -1.0, np.float32)
    for t in range(nt):
        dloc[:rows[t], t] = np.arange(rows[t], dtype=np.float32)
    dloc = dloc.astype(ml_dtypes.bfloat16)

    in_maps = []
    for c in range(NCORES):
        fpad = np.zeros((nt * PT, din0), np.float32)
        fpad[:nshard] = features[c * nshard:(c + 1) * nshard]
        im = dict(
            feat=fpad.astype(ml_dtypes.bfloat16),
            idx=per_core[c]["idx"],
            d1=per_core[c]["d1"],
            d2=per_core[c]["d2"],
            iota=iota,
            ident=ident,
            dloc=np.ascontiguousarray(dloc),
            invden=shard_scal(inv_den, c),
            invdst=shard_scal(inv_dst, c),
            invsrc=shard_scal(inv_src, c),
        )
        for k, w in enumerate(w_arrs):
            im[f"w{k}"] = w
        in_maps.append(im)
    return in_maps


def _run(features, src, dst, sage_ws, sage_bs, gc_ws, gc_bs, fc_w, fc_b):
    n_nodes, din0 = features.shape
    dims = [(w.shape[1], w.shape[0]) for w in sage_ws]
    fc_out = fc_w.shape[0]

    key = hashlib.sha1(
        np.asarray(src).tobytes() + np.asarray(dst).tobytes()
        + str((n_nodes, din0, dims, fc_out)).encode()
    ).hexdigest()
    if key in _CACHE:
        nc, meta, per_core = _CACHE[key]
    else:
        meta, per_core = _prep_graph(src, dst, n_nodes)
        nc = _build_program(meta, dims, fc_out)
        _CACHE[key] = (nc, meta, per_core)

    biases = list(sage_bs) + list(gc_bs) + [fc_b]
    if any(np.any(np.asarray(b) != 0) for b in biases):
        raise NotImplementedError("nonzero biases not supported")

    in_maps = _build_in_maps(features, src, dst, sage_ws, gc_ws, fc_w,
                             meta, per_core)

    trace = bool(int(os.environ.get("GNN_TRACE", "0")))
    res = bass_utils.run_bass_kernel_spmd(
        nc, in_maps, core_ids=list(range(NCORES)), trace=trace)
    out = np.concatenate([res.results[c]["out"] for c in range(NCORES)],
                         axis=0).astype(np.float32)
    if trace:
        print(f"[kernel] exec_time_ns: {res.exec_time_ns}", file=sys.stderr)
        _CACHE["last_exec_time_ns"] = res.exec_time_ns
        _CACHE["last_profile"] = res.profile_json
    return out


def kernel(features, src, dst,
           sage_w0, sage_b0, gc_w0, gc_b0,
           sage_w1, sage_b1, gc_w1, gc_b1,
           sage_w2, sage_b2, gc_w2, gc_b2,
           fc_w, fc_b):
    features = np.asarray(features, np.float32)
    src = np.asarray(src, np.int64)
    dst = np.asarray(dst, np.int64)
    return _run(
        features, src, dst,
        [np.asarray(sage_w0), np.asarray(sage_w1), np.asarray(sage_w2)],
        [np.asarray(sage_b0), np.asarray(sage_b1), np.asarray(sage_b2)],
        [np.asarray(gc_w0), np.asarray(gc_w1), np.asarray(gc_w2)],
        [np.asarray(gc_b0), np.asarray(gc_b1), np.asarray(gc_b2)],
        np.asarray(fc_w), np.asarray(fc_b),
    )


# revision 18
# speedup vs baseline: 1.6282x; 1.2566x over previous
"""GNN message-passing (SAGE-gcn + GraphConv stack) Trainium2 Bass kernel. V2

Strategy (8 NeuronCores, dst-node sharded):
- Each core owns a contiguous block of N/8 dst nodes and all edges into them.
- Per layer, the small weight matmul is applied BEFORE aggregation
  (A @ (h W^T) == (A @ h) W^T), shrinking the gathered message dim.
- Each core computes z for its own nodes; an in-kernel AllGather replicates
  the full [N, Dz] z table (bf16) into every core's DRAM.
- Aggregation: edges sorted by (src-half, dst-tile); gathered in uniform
  1024-row dma_gather calls spread over 4 SWDGE queues (SWDGE descriptor
  generation is the bottleneck resource). Chunks of 128 gathered rows are
  segment-summed into per-dst-tile PSUM via one-hot S matmuls; chunks may
  straddle dst-tile boundaries (one matmul per (chunk, tile) "job").
- Self-loops are removed from the gather stream. The sage "+h" term and the
  self-loop contribution ride a per-tile "local chunk" (contiguous HWDGE load
  of the core's own z rows) multiplied by an identity S column (twice for
  sage: 2*I accounts for self-loop + explicit h).
- Duplicate (src, dst-tile) edges are deduped: one gathered row can serve two
  dst positions via a second one-hot layer (d2) added to S on DVE.
- Host does index-only preprocessing: edge bucketing, dedup, int16 gather
  indices (two 25k src halves for int16 range), degree scalars.
"""
import os
import sys
import time
import hashlib

import numpy as np
import ml_dtypes

for _p in ("/opt/trn_rl_repo", "/root/.axon_site/_ro/trn_rl_repo"):
    if os.path.isdir(_p) and _p not in sys.path:
        sys.path.append(_p)

import concourse.bass as bass  # noqa: E402
import concourse.bacc as bacc  # noqa: E402
import concourse.mybir as mybir  # noqa: E402
from concourse import tile  # noqa: E402
from concourse import bass_utils  # noqa: E402

BF16 = mybir.dt.bfloat16
F32 = mybir.dt.float32
I16 = mybir.dt.int16

NCORES = 8
PT = 128
NQUEUES = 4
MAXC = 8  # chunks (x128 rows) per dma_gather call; 1024-idx HW cap


# ----------------------------------------------------------------------------
# host-side graph preprocessing (index-only)
# ----------------------------------------------------------------------------

def _wrap_idxs(idx_flat: np.ndarray) -> np.ndarray:
    """[n] int16 -> [128, n/16]: idx i at [i%16, i//16], replicated x8."""
    n = idx_flat.shape[0]
    assert n % 16 == 0
    w = idx_flat.reshape(n // 16, 16).T.astype(np.int16)
    return np.ascontiguousarray(np.tile(w, (8, 1)))


def _prep_graph(src: np.ndarray, dst: np.ndarray, n_nodes: int):
    """Bucket/dedup edges; build the uniform chunk/job layout.

    Returns (meta, per_core):
      meta: nshard, nt, half_n, nch=[c0,c1], jobs_by_tile (uniform),
            njobs, rows_ut [2][nt]
      per_core: idx [128, (c0+c1)*8] int16, d1/d2 [128, njobs] bf16
    """
    nshard = n_nodes // NCORES
    nt = (nshard + PT - 1) // PT
    half_n = n_nodes // 2
    assert half_n - 1 <= np.iinfo(np.int16).max

    src = np.asarray(src, np.int64)
    dst = np.asarray(dst, np.int64)
    nonself = src != dst
    src, dst = src[nonself], dst[nonself]

    # per core: rows[(h, t)] -> list of (src_local, m1, m2)
    core_rows = []
    for c in range(NCORES):
        mask = (dst >= c * nshard) & (dst < (c + 1) * nshard)
        es = src[mask]
        ed = dst[mask] - c * nshard
        h = (es >= half_n).astype(np.int64)
        t = ed >> 7
        m = ed & 127
        order = np.lexsort((m, es, t, h))
        es, t, m, h = es[order], t[order], m[order], h[order]
        rows = {}
        i = 0
        E = len(es)
        while i < E:
            # group run of identical (h, t, src)
            j = i + 1
            while j < E and es[j] == es[i] and t[j] == t[i] and h[j] == h[i]:
                j += 1
            key = (int(h[i]), int(t[i]))
            lst = rows.setdefault(key, [])
            sl = int(es[i] - h[i] * half_n)
            for k in range(i, j):
                lst.append((sl, int(m[k])))
            i = j
        core_rows.append(rows)

    # uniform per-cell row counts
    rows_ut = np.zeros((2, nt), np.int64)
    for h in range(2):
        for t in range(nt):
            rows_ut[h, t] = max(len(core_rows[c].get((h, t), []))
                                for c in range(NCORES))

    # chunk layout per half; tile 48 region extended to chunk pad
    off = np.zeros((2, nt), np.int64)
    nch = [0, 0]
    rows_ext = rows_ut.copy()
    for h in range(2):
        acc = 0
        for t in range(nt):
            off[h, t] = acc
            acc += rows_ut[h, t]
        nch[h] = (acc + PT - 1) // PT
        rows_ext[h, nt - 1] += nch[h] * PT - acc  # pad into last tile

    # jobs: per tile, ordered (h asc, chunk asc). job -> (h, c, t)
    jobs_by_tile = []
    njobs = 0
    for t in range(nt):
        jt = []
        for h in range(2):
            lo, hi = off[h, t], off[h, t] + rows_ext[h, t]
            for c in range(int(lo) >> 7, int(hi - 1 >> 7) + 1):
                jt.append((h, c, njobs))
                njobs += 1
        jobs_by_tile.append(jt)

    # per-core tables
    per_core = []
    for c in range(NCORES):
        idx_flat = np.zeros((nch[0] + nch[1]) * PT, np.int16)
        d1 = np.full((PT, njobs), -1.0, np.float32)
        rowbuf = {}
        for h in range(2):
            for t in range(nt):
                lst = core_rows[c].get((h, t), [])
                base = (nch[0] * PT if h else 0) + int(off[h, t])
                for r, (sl, m1) in enumerate(lst):
                    idx_flat[base + r] = sl
                    rowbuf[base + r] = m1
        for t in range(nt):
            for (h, ch, j) in jobs_by_tile[t]:
                gbase = (nch[0] * PT if h else 0) + ch * PT
                lo = (nch[0] * PT if h else 0) + int(off[h, t])
                hi = lo + int(rows_ut[h, t])
                for p in range(PT):
                    g = gbase + p
                    if lo <= g < hi and g in rowbuf:
                        d1[p, j] = float(rowbuf[g])
        per_core.append(dict(
            idx=_wrap_idxs(idx_flat),
            d1=np.ascontiguousarray(d1.astype(ml_dtypes.bfloat16)),
        ))

    meta = dict(nshard=nshard, nt=nt, half_n=half_n, nch=nch,
                jobs_by_tile=jobs_by_tile, njobs=njobs, rows_ut=rows_ut)
    return meta, per_core


# ----------------------------------------------------------------------------
# device program builder
# ----------------------------------------------------------------------------

def _build_program(meta, dims, fc_out):
    nshard, nt, half_n = meta["nshard"], meta["nt"], meta["half_n"]
    nch = meta["nch"]
    jobs_by_tile, njobs = meta["jobs_by_tile"], meta["njobs"]
    nchunks = nch[0] + nch[1]
    din0 = dims[0][0]

    wdims = []
    for (di, do) in dims:
        wdims.append((di, do))   # sage
        wdims.append((do, do))   # gc
    wdims.append((dims[-1][1], fc_out))  # fc
    agg_dz = [wdims[k][1] for k in range(6)]
    max_dz = max(agg_dz)
    max_jobs_t = max(len(j) for j in jobs_by_tile)

    rows = [PT] * nt
    rows[nt - 1] = nshard - (nt - 1) * PT
    npad = nt * PT  # padded z_own rows

    nc = bacc.Bacc("TRN2", target_bir_lowering=False, debug=False,
                   num_devices=NCORES, num_swdge_queues=NQUEUES)

    feat = nc.dram_tensor("feat", [npad, din0], BF16, kind="ExternalInput")
    idx_in = nc.dram_tensor("idx", [PT, nchunks * 8], I16,
                            kind="ExternalInput")
    d1_in = nc.dram_tensor("d1", [PT, njobs], BF16, kind="ExternalInput")
    iota_in = nc.dram_tensor("iota", [PT, PT], BF16, kind="ExternalInput")
    ident_in = nc.dram_tensor("ident", [PT, PT], BF16, kind="ExternalInput")
    dloc_in = nc.dram_tensor("dloc", [PT, nt], BF16, kind="ExternalInput")
    invden_in = nc.dram_tensor("invden", [PT, nt], F32, kind="ExternalInput")
    invdst_in = nc.dram_tensor("invdst", [PT, nt], F32, kind="ExternalInput")
    invsrc_in = nc.dram_tensor("invsrc", [PT, nt], F32, kind="ExternalInput")
    w_in = []
    for k, (di, do) in enumerate(wdims):
        w_in.append(nc.dram_tensor(f"w{k}", [PT, di // PT, do], BF16,
                                   kind="ExternalInput"))
    out = nc.dram_tensor("out", [nshard, fc_out], F32, kind="ExternalOutput")

    with tile.TileContext(nc) as tcx:
        with (
            tcx.tile_pool(name="const", bufs=1) as constp,
            tcx.tile_pool(name="g0", bufs=5) as g0p,
            tcx.tile_pool(name="g1", bufs=5) as g1p,
            tcx.tile_pool(name="zt", bufs=2) as ztp,
            tcx.tile_pool(name="s", bufs=2) as sp,
            tcx.tile_pool(name="h", bufs=3) as hp,
            tcx.tile_pool(name="ht", bufs=2) as htp,
            tcx.tile_pool(name="zo", bufs=3) as zop,
            tcx.tile_pool(name="aggps", bufs=2, space="PSUM") as aggpsp,
            tcx.tile_pool(name="trps", bufs=2, space="PSUM") as trpsp,
            tcx.tile_pool(name="mmps", bufs=2, space="PSUM") as mmpsp,
            tcx.tile_pool(name="dram", bufs=1, space="DRAM") as dramp,
        ):
            # persistent constants
            idx_sb = constp.tile([PT, nchunks * 8], I16, name="idx_sb")
            nc.sync.dma_start(idx_sb[:], idx_in[:])
            d1_sb = constp.tile([PT, njobs], BF16, name="d1_sb")
            nc.sync.dma_start(d1_sb[:], d1_in[:])
            iota_sb = constp.tile([PT, PT], BF16, name="iota_sb")
            nc.sync.dma_start(iota_sb[:], iota_in[:])
            ident_sb = constp.tile([PT, PT], BF16, name="ident_sb")
            nc.sync.dma_start(ident_sb[:], ident_in[:])
            dloc_sb = constp.tile([PT, nt], BF16, name="dloc_sb")
            nc.sync.dma_start(dloc_sb[:], dloc_in[:])
            invden_sb = constp.tile([PT, nt], F32, name="invden_sb")
            nc.sync.dma_start(invden_sb[:], invden_in[:])
            invdst_sb = constp.tile([PT, nt], F32, name="invdst_sb")
            nc.sync.dma_start(invdst_sb[:], invdst_in[:])
            invsrc_sb = constp.tile([PT, nt], F32, name="invsrc_sb")
            nc.sync.dma_start(invsrc_sb[:], invsrc_in[:])
            w_sb = []
            for k, (di, do) in enumerate(wdims):
                w = constp.tile([PT, di // PT, do], BF16, name=f"w{k}_sb")
                nc.sync.dma_start(w[:], w_in[k][:])
                w_sb.append(w)

            # per-tile local identity columns: ls[:, t, :] = I masked to rows[t]
            ls_sb = constp.tile([PT, nt, PT], BF16, name="ls_sb")
            iota_bnt = iota_sb[:].unsqueeze(1).broadcast_to([PT, nt, PT])
            dloc_bnt = dloc_sb[:].unsqueeze(2).broadcast_to([PT, nt, PT])
            nc.vector.tensor_tensor(ls_sb[:], iota_bnt, dloc_bnt,
                                    mybir.AluOpType.is_equal)

            # internal DRAM: z shards (padded) + gathered full tables
            z_own, z_full = [], []
            for a in range(6):
                dz = agg_dz[a]
                z_own.append(dramp.tile([npad, dz], BF16, name=f"zown{a}"))
                z_full.append(dramp.tile([NCORES * nshard, dz], BF16,
                                         name=f"zfull{a}",
                                         addr_space="Shared"))

            def run_ag(a):
                nc.gpsimd.collective_compute(
                    "AllGather", mybir.AluOpType.bypass,
                    replica_groups=[list(range(NCORES))],
                    ins=[z_own[a][:nshard, :].opt()],
                    outs=[z_full[a][:].opt()],
                )

            def stage_b(t, h_tile, k, dest, dest_dtype, prescale):
                """h_tile [128, Din_k] bf16 -> dest rows of tile t."""
                di, do = wdims[k]
                kg = di // PT
                ht = htp.tile([PT, kg, PT], BF16, tag="ht")
                for g in range(kg):
                    trp = trpsp.tile([PT, PT], BF16, tag="tr")
                    nc.tensor.transpose(trp[:], h_tile[:, g * PT:(g + 1) * PT],
                                        ident_sb[:])
                    nc.vector.tensor_copy(ht[:, g, :], trp[:])
                zp = mmpsp.tile([PT, do], F32, tag="mm")
                for g in range(kg):
                    nc.tensor.matmul(zp[:], ht[:, g, :], w_sb[k][:, g, :],
                                     start=(g == 0), stop=(g == kg - 1))
                zo = zop.tile([PT, do], dest_dtype, tag="zo")
                if prescale is not None:
                    nc.scalar.activation(zo[:], zp[:],
                                         mybir.ActivationFunctionType.Copy,
                                         scale=prescale[:, t:t + 1])
                else:
                    nc.vector.tensor_copy(zo[:], zp[:])
                if dest_dtype == F32:  # fc output: unpadded dest
                    nc.sync.dma_start(dest[t * PT:t * PT + rows[t], :],
                                      zo[:rows[t], :])
                else:
                    nc.sync.dma_start(dest[t * PT:(t + 1) * PT, :], zo[:])

            # ---- stage B0: features -> z0 ----
            for t in range(nt):
                h0 = hp.tile([PT, din0], BF16, tag="h")
                nc.sync.dma_start(h0[:], feat[t * PT:(t + 1) * PT, :])
                stage_b(t, h0, 0, z_own[0], BF16, None)
            run_ag(0)

            # ---- 6 aggregation layers + following stage B ----
            call_id = 0
            for a in range(6):
                dz = agg_dz[a]
                sage = (a % 2 == 0)
                k_next = a + 1
                is_fc = (k_next == 6)

                gbufs = [{}, {}]  # per half: call -> g tile
                cursor = [0, 0]   # next call to emit, per half
                ncalls = [(nch[0] + MAXC - 1) // MAXC,
                          (nch[1] + MAXC - 1) // MAXC]

                def emit_calls(h, upto_chunk):
                    nonlocal call_id
                    gp = g0p if h == 0 else g1p
                    half = z_full[a][h * half_n:(h + 1) * half_n, :]
                    while (cursor[h] * MAXC <= upto_chunk
                           and cursor[h] < ncalls[h]):
                        ci = cursor[h]
                        cw = min(MAXC, nch[h] - ci * MAXC)
                        g = gp.tile([PT, MAXC, dz], BF16, tag=f"g{h}")
                        base = (nch[0] if h else 0) + ci * MAXC
                        nc.gpsimd.dma_gather(
                            g[:, :cw, :], half,
                            idx_sb[:, base * 8:(base + cw) * 8],
                            cw * PT, cw * PT, dz,
                            queue_num=call_id % NQUEUES)
                        gbufs[h][ci] = g
                        cursor[h] += 1
                        call_id += 1

                for t in range(nt):
                    jt = jobs_by_tile[t]
                    nj = len(jt)
                    j0 = jt[0][2]
                    assert [j for (_, _, j) in jt] == list(range(j0, j0 + nj))
                    for h in (0, 1):
                        hc = [c for (hh, c, _) in jt if hh == h]
                        if hc:
                            emit_calls(h, max(hc))
                    # local chunk (self-loop + sage's explicit +h)
                    zt = ztp.tile([PT, dz], BF16, tag="zt")
                    nc.sync.dma_start(zt[:],
                                      z_own[a][t * PT:(t + 1) * PT, :])
                    # S for gather jobs: one-hot of d1
                    s = sp.tile([PT, nj, PT], BF16, tag="s")
                    iota_b = iota_sb[:].unsqueeze(1).broadcast_to([PT, nj, PT])
                    d1_b = d1_sb[:, j0:j0 + nj].unsqueeze(2) \
                        .broadcast_to([PT, nj, PT])
                    nc.vector.tensor_tensor(s[:], iota_b, d1_b,
                                            mybir.AluOpType.is_equal)
                    # matmuls: local first (x2 for sage), then gather jobs
                    aggp = aggpsp.tile([PT, dz], F32, tag="agg")
                    nc.tensor.matmul(aggp[:], ls_sb[:, t, :], zt[:],
                                     start=True, stop=False)
                    if sage:
                        nc.tensor.matmul(aggp[:], ls_sb[:, t, :], zt[:],
                                         start=False, stop=False)
                    for ji, (h, c, j) in enumerate(jt):
                        g = gbufs[h][c // MAXC]
                        nc.tensor.matmul(aggp[:], s[:, ji, :],
                                         g[:, c % MAXC, :],
                                         start=False, stop=(ji == nj - 1))
                    # epilogue: relu(scale * agg)
                    hx = hp.tile([PT, dz], BF16, tag="h")
                    scl = invden_sb if sage else invdst_sb
                    nc.scalar.activation(hx[:], aggp[:],
                                         mybir.ActivationFunctionType.Relu,
                                         scale=scl[:, t:t + 1])
                    # next weight stage
                    if is_fc:
                        stage_b(t, hx, 6, out, F32, None)
                    else:
                        prescale = invsrc_sb if sage else None
                        stage_b(t, hx, k_next, z_own[k_next], BF16, prescale)
                if not is_fc:
                    run_ag(k_next)

    t0 = time.time()
    nc.compile()
    print(f"[kernel] bacc compile: {time.time() - t0:.1f}s", file=sys.stderr)
    return nc


# ----------------------------------------------------------------------------
# public entry
# ----------------------------------------------------------------------------

_CACHE = {}


def _build_in_maps(features, src, dst, sage_ws, gc_ws, fc_w, meta, per_core):
    n_nodes, din0 = features.shape
    nshard, nt = meta["nshard"], meta["nt"]

    e_ones = np.ones(len(src), np.float64)
    in_deg = np.bincount(dst, weights=e_ones, minlength=n_nodes)
    out_deg = np.bincount(src, weights=e_ones, minlength=n_nodes)
    inv_den = (1.0 / (in_deg + 1.0)).astype(np.float32)
    inv_dst = (np.where(in_deg > 0, in_deg, 1.0) ** -0.5).astype(np.float32)
    inv_src = (np.where(out_deg > 0, out_deg, 1.0) ** -0.5).astype(np.float32)

    def shard_scal(v, c):
        s = np.ones(nt * PT, np.float32)
        s[:nshard] = v[c * nshard:(c + 1) * nshard]
        return np.ascontiguousarray(s.reshape(nt, PT).T)

    worder = []
    for s in range(len(sage_ws)):
        worder.append(sage_ws[s])
        worder.append(gc_ws[s])
    worder.append(fc_w)
    w_arrs = []
    for w in worder:
        do, di = w.shape
        wt = np.ascontiguousarray(
            w.T.astype(np.float32).reshape(di // PT, PT, do)
            .transpose(1, 0, 2)).astype(ml_dtypes.bfloat16)
        w_arrs.append(wt)

    iota = np.tile(np.arange(PT, dtype=np.float32), (PT, 1)) \
        .astype(ml_dtypes.bfloat16)
    ident = np.eye(PT, dtype=np.float32).astype(ml_dtypes.bfloat16)
    # dloc[:, t] = partition index if < rows[t] else -1
    rows = [PT] * nt
    rows[nt - 1] = nshard - (nt - 1) * PT
    dloc = np.full((PT, nt), -1.0, np.float32)
    for t in range(nt):
        dloc[:rows[t], t] = np.arange(rows[t], dtype=np.float32)
    dloc = dloc.astype(ml_dtypes.bfloat16)

    in_maps = []
    for c in range(NCORES):
        fpad = np.zeros((nt * PT, din0), np.float32)
        fpad[:nshard] = features[c * nshard:(c + 1) * nshard]
        im = dict(
            feat=fpad.astype(ml_dtypes.bfloat16),
            idx=per_core[c]["idx"],
            d1=per_core[c]["d1"],
            iota=iota,
            ident=ident,
            dloc=np.ascontiguousarray(dloc),
            invden=shard_scal(inv_den, c),
            invdst=shard_scal(inv_dst, c),
            invsrc=shard_scal(inv_src, c),
        )
        for k, w in enumerate(w_arrs):
            im[f"w{k}"] = w
        in_maps.append(im)
    return in_maps


def _run(features, src, dst, sage_ws, sage_bs, gc_ws, gc_bs, fc_w, fc_b):
    n_nodes, din0 = features.shape
    dims = [(w.shape[1], w.shape[0]) for w in sage_ws]
    fc_out = fc_w.shape[0]

    key = hashlib.sha1(
        np.asarray(src).tobytes() + np.asarray(dst).tobytes()
        + str((n_nodes, din0, dims, fc_out)).encode()
    ).hexdigest()
    if key in _CACHE:
        nc, meta, per_core = _CACHE[key]
    else:
        meta, per_core = _prep_graph(src, dst, n_nodes)
        nc = _build_program(meta, dims, fc_out)
        _CACHE[key] = (nc, meta, per_core)

    biases = list(sage_bs) + list(gc_bs) + [fc_b]
    if any(np.any(np.asarray(b) != 0) for b in biases):
        raise NotImplementedError("nonzero biases not supported")

    in_maps = _build_in_maps(features, src, dst, sage_ws, gc_ws, fc_w,
                             meta, per_core)

    trace = bool(int(os.environ.get("GNN_TRACE", "0")))
    res = bass_utils.run_bass_kernel_spmd(
        nc, in_maps, core_ids=list(range(NCORES)), trace=trace)
    out = np.concatenate([res.results[c]["out"] for c in range(NCORES)],
                         axis=0).astype(np.float32)
    if trace:
        print(f"[kernel] exec_time_ns: {res.exec_time_ns}", file=sys.stderr)
        _CACHE["last_exec_time_ns"] = res.exec_time_ns
        _CACHE["last_profile"] = res.profile_json
    return out


def kernel(features, src, dst,
           sage_w0, sage_b0, gc_w0, gc_b0,
           sage_w1, sage_b1, gc_w1, gc_b1,
           sage_w2, sage_b2, gc_w2, gc_b2,
           fc_w, fc_b):
    features = np.asarray(features, np.float32)
    src = np.asarray(src, np.int64)
    dst = np.asarray(dst, np.int64)
    return _run(
        features, src, dst,
        [np.asarray(sage_w0), np.asarray(sage_w1), np.asarray(sage_w2)],
        [np.asarray(sage_b0), np.asarray(sage_b1), np.asarray(sage_b2)],
        [np.asarray(gc_w0), np.asarray(gc_w1), np.asarray(gc_w2)],
        [np.asarray(gc_b0), np.asarray(gc_b1), np.asarray(gc_b2)],
        np.asarray(fc_w), np.asarray(fc_b),
    )
